# revision 14
# baseline (speedup 1.0000x reference)
"""Trainium2 Bass kernel for the mixture-of-tastes edge scoring model.

y[b] = sum_m softmax_m(A[u_b] @ e[v_b]) * (U[u_b] @ e[v_b]) + ub[u_b] + mb[v_b]

Strategy: data-parallel over the 524288 edges across 8 NeuronCores.  The
embedding tables are packed on the host into gather-friendly rows:

  user_packed[u]  = [attn(8x32) | taste'(8x34) | pad]   (576 f32 = 2304 B)
      taste'[m] = [taste[m] (32) | user_bias[u] | 1.0]
  movie_packed[v] = [movie_emb[v] (32) | 1.0 | movie_bias[v] | pad] (64 f32)

With e'' = movie_packed[v][0:34] = [e, 1, mb], the fold
  U'_m . e'' = U_m . e + ub + mb
adds (ub+mb) to every score; since softmax weights sum to 1 this adds
(ub+mb) to the output, so no separate bias gather is needed.  Softmax is
computed without max subtraction (logits are O(1e-2) here, exp is safe).

Each core loops over 2048-edge chunks: two SWDGE dma_gathers (user rows
2304 B, movie rows 256 B), then DVE broadcast-multiply + 3D-AP reduce for
logits/scores, ACT exp, DVE weighted combine.  Gather order is permuted so
the per-core output lands as a contiguous [128, 512] SBUF tile.
"""

import sys

sys.path.insert(0, "/opt/trn_rl_repo")

import numpy as np

import concourse.bacc as bacc
import concourse.bass as bass
import concourse.mybir as mybir
from concourse.bass_utils import run_bass_kernel_spmd
from concourse.tile import TileContext

# Problem constants (nn_MoT_43533788512463)
B = 524288
N_CORES = 8
M, K = 8, 32
N_ROWS = 20000  # edge indices are randint(0, 20000) per the spec
UROW = 576  # packed user row: 256 attn + 272 taste' + 48 pad (f32)
VROW = 64  # packed movie row: 32 e + 1.0 + mb + 30 pad (f32)
P = 128
CHUNK = 1024  # indices per dma_gather (2048 overflows the SWDGE packet limit and faults)
NBLK = CHUNK // P  # column blocks per chunk
IDXW = CHUNK // 16  # idx-tile columns per chunk (16-partition wrap)

F32 = mybir.dt.float32
I16 = mybir.dt.int16
MULT = mybir.AluOpType.mult
ADD = mybir.AluOpType.add
AX_X = mybir.AxisListType.X


def build_nc(edges_per_core: int = B // N_CORES) -> bass.Bass:
    """One NeuronCore's program; SPMD across cores with different inputs."""
    epc = edges_per_core
    assert epc % CHUNK == 0
    nchunk = epc // CHUNK
    cols = epc // P  # output columns per partition

    nc = bacc.Bacc("TRN2", debug=False)
    user_d = nc.dram_tensor("user_packed", [N_ROWS, UROW], F32, kind="ExternalInput")
    movie_d = nc.dram_tensor("movie_packed", [N_ROWS, VROW], F32, kind="ExternalInput")
    idx_d = nc.dram_tensor(
        "idx_uv", [P, 2 * nchunk * IDXW], I16, kind="ExternalInput"
    )
    y_d = nc.dram_tensor("y", [P, cols], F32, kind="ExternalOutput")

    with TileContext(nc) as tc:
        with (
            tc.tile_pool(name="persist", bufs=1) as pp,
            tc.tile_pool(name="io", bufs=3) as iop,
            tc.tile_pool(name="mid", bufs=2) as midp,
        ):
            idxs = pp.tile([P, 2 * nchunk * IDXW], I16)
            nc.sync.dma_start(idxs[:, :], idx_d[:, :])
            ysb = pp.tile([P, cols], F32)

            for c in range(nchunk):
                du = iop.tile([P, NBLK, UROW], F32, tag="du")
                dv = iop.tile([P, NBLK, VROW], F32, tag="dv")
                usl = idxs[:, c * IDXW : (c + 1) * IDXW]
                vsl = idxs[:, (nchunk + c) * IDXW : (nchunk + c + 1) * IDXW]
                nc.gpsimd.dma_gather(
                    du[:, :, :], user_d[:, :], usl, CHUNK, CHUNK, UROW
                )
                nc.gpsimd.dma_gather(
                    dv[:, :, :], movie_d[:, :], vsl, CHUNK, CHUNK, VROW
                )

                a4 = du[:, :, 0:256].rearrange("p b (m k) -> p b m k", m=M)
                u4 = du[:, :, 256:528].rearrange("p b (m k) -> p b m k", m=M)
                e32 = dv[:, :, 0:K].unsqueeze(2).broadcast_to([P, NBLK, M, K])
                e34 = (
                    dv[:, :, 0 : K + 2]
                    .unsqueeze(2)
                    .broadcast_to([P, NBLK, M, K + 2])
                )

                prod_a = midp.tile([P, NBLK, M, K], F32, tag="prod_a")
                prod_u = midp.tile([P, NBLK, M, K + 2], F32, tag="prod_u")
                logits = midp.tile([P, NBLK, M], F32, tag="logits")
                scores = midp.tile([P, NBLK, M], F32, tag="scores")
                exps = midp.tile([P, NBLK, M], F32, tag="exps")
                wprod = midp.tile([P, NBLK, M], F32, tag="wprod")
                num_t = midp.tile([P, NBLK], F32, tag="num_t")
                den_t = midp.tile([P, NBLK], F32, tag="den_t")
                rden_t = midp.tile([P, NBLK], F32, tag="rden_t")

                nc.vector.tensor_tensor(prod_a[:, :, :, :], a4, e32, op=MULT)
                nc.vector.tensor_reduce(
                    logits[:, :, :], prod_a[:, :, :, :], AX_X, ADD
                )
                nc.vector.tensor_tensor(prod_u[:, :, :, :], u4, e34, op=MULT)
                nc.vector.tensor_reduce(
                    scores[:, :, :], prod_u[:, :, :, :], AX_X, ADD
                )
                nc.scalar.activation(
                    exps[:, :, :],
                    logits[:, :, :],
                    mybir.ActivationFunctionType.Exp,
                )
                nc.vector.tensor_tensor(
                    wprod[:, :, :], exps[:, :, :], scores[:, :, :], op=MULT
                )
                nc.vector.tensor_reduce(num_t[:, :], wprod[:, :, :], AX_X, ADD)
                nc.vector.tensor_reduce(den_t[:, :], exps[:, :, :], AX_X, ADD)
                nc.vector.reciprocal(rden_t[:, :], den_t[:, :])
                nc.vector.tensor_tensor(
                    ysb[:, c * NBLK : (c + 1) * NBLK],
                    num_t[:, :],
                    rden_t[:, :],
                    op=MULT,
                )

            nc.sync.dma_start(y_d[:, :], ysb[:, :])

    nc.compile()
    return nc


def pack_tables(taste_emb, attn_emb, movie_emb, user_bias, movie_bias):
    taste_emb = np.asarray(taste_emb, dtype=np.float32)
    attn_emb = np.asarray(attn_emb, dtype=np.float32)
    movie_emb = np.asarray(movie_emb, dtype=np.float32)
    user_bias = np.asarray(user_bias, dtype=np.float32)
    movie_bias = np.asarray(movie_bias, dtype=np.float32)

    nr = N_ROWS
    ublk = np.zeros((nr, M, K + 2), np.float32)
    ublk[:, :, :K] = taste_emb[:nr].reshape(nr, M, K)
    ublk[:, :, K] = user_bias[:nr, 0][:, None]
    ublk[:, :, K + 1] = 1.0
    user_packed = np.zeros((nr, UROW), np.float32)
    user_packed[:, 0:256] = attn_emb[:nr]
    user_packed[:, 256:528] = ublk.reshape(nr, 272)

    nm = movie_emb.shape[0]
    assert nm <= N_ROWS
    movie_packed = np.zeros((N_ROWS, VROW), np.float32)
    movie_packed[:nm, :K] = movie_emb
    movie_packed[:nm, K] = 1.0
    movie_packed[:nm, K + 1] = movie_bias[:, 0]
    return user_packed, movie_packed


def make_idx_tile(idx_logical: np.ndarray, nchunk: int) -> np.ndarray:
    """Wrap a logical gather-order int sequence into dma_gather's SBUF layout:
    [128, nchunk*IDXW] int16; per chunk [16, IDXW] wrapped + replicated x8."""
    w = idx_logical.astype(np.int16).reshape(nchunk, IDXW, 16).transpose(0, 2, 1)
    w = np.tile(w, (1, P // 16, 1))  # [nchunk, 128, IDXW]
    return np.ascontiguousarray(w.transpose(1, 0, 2).reshape(P, nchunk * IDXW))


_NC_CACHE: dict[int, bass.Bass] = {}


def prepare_in_maps(edge, taste_emb, attn_emb, movie_emb, user_bias, movie_bias):
    edge = np.asarray(edge)
    u = edge[:, 0].astype(np.int64)
    v = edge[:, 1].astype(np.int64)
    b = edge.shape[0]
    assert b == B and b % N_CORES == 0
    assert u.max() < N_ROWS and v.max() < N_ROWS

    epc = b // N_CORES
    cols = epc // P
    nchunk = epc // CHUNK

    user_packed, movie_packed = pack_tables(
        taste_emb, attn_emb, movie_emb, user_bias, movie_bias
    )

    # gather position i lands at out[i%128, i//128]; choose perm so that
    # out[p, c] = local edge p*cols + c (contiguous per-partition output)
    ar = np.arange(epc)
    perm = (ar % P) * cols + ar // P

    in_maps = []
    for r in range(N_CORES):
        ul = u[r * epc : (r + 1) * epc][perm]
        vl = v[r * epc : (r + 1) * epc][perm]
        in_maps.append(
            {
                "user_packed": user_packed,
                "movie_packed": movie_packed,
                "idx_uv": np.concatenate(
                    [make_idx_tile(ul, nchunk), make_idx_tile(vl, nchunk)], axis=1
                ),
            }
        )
    return in_maps, epc


def run(in_maps, epc, **kwargs):
    if epc not in _NC_CACHE:
        _NC_CACHE[epc] = build_nc(epc)
    nc = _NC_CACHE[epc]
    return run_bass_kernel_spmd(nc, in_maps, core_ids=list(range(N_CORES)), **kwargs)


def kernel(edge, taste_emb, attn_emb, movie_emb, user_bias, movie_bias):
    in_maps, epc = prepare_in_maps(
        edge, taste_emb, attn_emb, movie_emb, user_bias, movie_bias
    )
    res = run(in_maps, epc)
    return np.concatenate([res.results[r]["y"].reshape(-1) for r in range(N_CORES)])


# revision 15
# speedup vs baseline: 1.4100x; 1.4100x over previous
"""Trainium2 Bass kernel for the mixture-of-tastes edge scoring model.

y[b] = sum_m softmax_m(A[u_b] @ e[v_b]) * (U[u_b] @ e[v_b]) + ub[u_b] + mb[v_b]

The kernel is gather-descriptor-bound on TRN2 (the Q7 SWDGE generates
descriptors at ~8-10 ns each), so the layout is built to minimize
descriptor count:

- Edges are partitioned across the 8 cores BY USER RANGE (user u goes to
  core u // 2500), so each user's ~26 edges land on one core.  Each core's
  edges are grouped by user into groups of G=8 slots (padded with dummy
  slots), so ONE user-row gather descriptor serves 8 edges.
- Movie rows are gathered per slot (unavoidable: 1 descriptor each).
- Group j maps to (partition j%128, output column block j//128); slot s of
  group j is output element [j%128, (j//128)*8 + s].  The host keeps a
  slot->edge map and unscatters at the end (dummy slots dropped).

Tables are packed on the host into gather-friendly bf16 rows (bf16 also
gives the DVE its 2x 16-bit mode):

  user_packed[u]  = [attn(8x32) | taste'(8x34) | pad]  (640 bf16 = 1280 B)
      taste'[m] = [taste[m] (32) | user_bias[u] | 1.0]
  movie_packed[v] = [e (32) | 1.0 | mb | pad]          (128 bf16 = 256 B)

With e'' = movie_packed[v][0:34] = [e, 1, mb], the fold
  U'_m . e'' = U_m . e + ub + mb
adds (ub+mb) to every score; softmax weights sum to 1, so the output gets
+(ub+mb) with no separate bias gather.  Softmax is computed without max
subtraction (logits are O(1e-2) here; exp cannot overflow).

Per 1024-slot chunk: one movie dma_gather + DVE broadcast-multiply
(user rows broadcast over the 8 slots of their group) + 3D-AP reduces,
ACT exp, DVE weighted combine.  One 1024-group user dma_gather feeds 8
chunks.
"""

import sys

sys.path.insert(0, "/opt/trn_rl_repo")

import ml_dtypes
import numpy as np

import concourse.bacc as bacc
import concourse.bass as bass
import concourse.mybir as mybir
from concourse.bass_utils import run_bass_kernel_spmd
from concourse.tile import TileContext

# Problem constants (nn_MoT_43533788512463)
B = 524288
N_CORES = 8
M, K = 8, 32
N_ROWS = 20000  # edge indices are randint(0, 20000) per the spec
UPC = N_ROWS // N_CORES  # users per core (u-range partitioning)
G = 8  # slots (edges) per user group
UROW = 640  # packed user row bf16: 256 attn + 272 taste' + 112 pad
VROW = 128  # packed movie row bf16: 32 e + 1.0 + mb + 94 pad
P = 128
CHUNK = 1024  # slots per movie gather / compute chunk
NBLK = CHUNK // P  # 8 column blocks per chunk
GPC = CHUNK  # groups per user gather (1024 groups = 8 chunks)

# Per-core slot capacity.  Expected need: 2500 users x E[ceil(n/8)*8]
# (n ~ Poisson(26.2)) ~= 76600; 81920 leaves ~6% slack.
N_CHUNKS = 80
CAP = N_CHUNKS * CHUNK  # 81920 slots
N_SC = N_CHUNKS // G  # 10 user-gather super-chunks
COLS = CAP // P  # 640 output columns per partition

BF16 = mybir.dt.bfloat16
F32 = mybir.dt.float32
I16 = mybir.dt.int16
MULT = mybir.AluOpType.mult
ADD = mybir.AluOpType.add
AX_X = mybir.AxisListType.X


def build_nc() -> bass.Bass:
    """One NeuronCore's program; SPMD across cores with different inputs."""
    nc = bacc.Bacc("TRN2", debug=False)
    user_d = nc.dram_tensor("user_packed", [N_ROWS, UROW], BF16, kind="ExternalInput")
    movie_d = nc.dram_tensor("movie_packed", [N_ROWS, VROW], BF16, kind="ExternalInput")
    # user idx: N_SC gathers x (GPC/16) cols; movie idx: N_CHUNKS x (CHUNK/16)
    uw, vw = GPC // 16, CHUNK // 16
    idx_d = nc.dram_tensor(
        "idx_uv", [P, N_SC * uw + N_CHUNKS * vw], I16, kind="ExternalInput"
    )
    y_d = nc.dram_tensor("y", [P, COLS], F32, kind="ExternalOutput")

    with TileContext(nc) as tc:
        with (
            tc.tile_pool(name="persist", bufs=1) as pp,
            tc.tile_pool(name="io", bufs=3) as iop,
            tc.tile_pool(name="mid", bufs=2) as midp,
        ):
            idxs = pp.tile([P, N_SC * uw + N_CHUNKS * vw], I16)
            nc.sync.dma_start(idxs[:, :], idx_d[:, :])
            ysb = pp.tile([P, COLS], F32)

            for sc in range(N_SC):
                us = iop.tile([P, G, UROW], BF16, tag="us")
                usl = idxs[:, sc * uw : (sc + 1) * uw]
                nc.gpsimd.dma_gather(
                    us[:, :, :], user_d[:, :], usl, GPC, GPC, UROW
                )
                for cc in range(G):
                    c = sc * G + cc
                    mv = iop.tile([P, NBLK, VROW], BF16, tag="mv")
                    vsl = idxs[
                        :, N_SC * uw + c * vw : N_SC * uw + (c + 1) * vw
                    ]
                    nc.gpsimd.dma_gather(
                        mv[:, :, :], movie_d[:, :], vsl, CHUNK, CHUNK, VROW
                    )

                    # group's user row broadcast over its 8 slots (dim 1);
                    # slot's movie row broadcast over the 8 tastes (dim 2)
                    a4 = (
                        us[:, cc, 0:256]
                        .rearrange("p (m k) -> p m k", m=M)
                        .unsqueeze(1)
                        .broadcast_to([P, NBLK, M, K])
                    )
                    u4 = (
                        us[:, cc, 256:528]
                        .rearrange("p (m k) -> p m k", m=M)
                        .unsqueeze(1)
                        .broadcast_to([P, NBLK, M, K + 2])
                    )
                    e32 = (
                        mv[:, :, 0:K].unsqueeze(2).broadcast_to([P, NBLK, M, K])
                    )
                    e34 = (
                        mv[:, :, 0 : K + 2]
                        .unsqueeze(2)
                        .broadcast_to([P, NBLK, M, K + 2])
                    )

                    prod_a = midp.tile([P, NBLK, M, K], BF16, tag="prod_a")
                    prod_u = midp.tile([P, NBLK, M, K + 2], BF16, tag="prod_u")
                    logits = midp.tile([P, NBLK, M], F32, tag="logits")
                    scores = midp.tile([P, NBLK, M], F32, tag="scores")
                    exps = midp.tile([P, NBLK, M], F32, tag="exps")
                    wprod = midp.tile([P, NBLK, M], F32, tag="wprod")
                    num_t = midp.tile([P, NBLK], F32, tag="num_t")
                    den_t = midp.tile([P, NBLK], F32, tag="den_t")
                    rden_t = midp.tile([P, NBLK], F32, tag="rden_t")

                    nc.vector.tensor_tensor(prod_a[:, :, :, :], a4, e32, op=MULT)
                    nc.vector.tensor_reduce(
                        logits[:, :, :], prod_a[:, :, :, :], AX_X, ADD
                    )
                    nc.vector.tensor_tensor(prod_u[:, :, :, :], u4, e34, op=MULT)
                    nc.vector.tensor_reduce(
                        scores[:, :, :], prod_u[:, :, :, :], AX_X, ADD
                    )
                    nc.scalar.activation(
                        exps[:, :, :],
                        logits[:, :, :],
                        mybir.ActivationFunctionType.Exp,
                    )
                    nc.vector.tensor_tensor(
                        wprod[:, :, :], exps[:, :, :], scores[:, :, :], op=MULT
                    )
                    nc.vector.tensor_reduce(num_t[:, :], wprod[:, :, :], AX_X, ADD)
                    nc.vector.tensor_reduce(den_t[:, :], exps[:, :, :], AX_X, ADD)
                    nc.vector.reciprocal(rden_t[:, :], den_t[:, :])
                    nc.vector.tensor_tensor(
                        ysb[:, c * NBLK : (c + 1) * NBLK],
                        num_t[:, :],
                        rden_t[:, :],
                        op=MULT,
                    )

            nc.sync.dma_start(y_d[:, :], ysb[:, :])

    nc.compile()
    return nc


def pack_tables(taste_emb, attn_emb, movie_emb, user_bias, movie_bias):
    taste_emb = np.asarray(taste_emb, dtype=np.float32)
    attn_emb = np.asarray(attn_emb, dtype=np.float32)
    movie_emb = np.asarray(movie_emb, dtype=np.float32)
    user_bias = np.asarray(user_bias, dtype=np.float32)
    movie_bias = np.asarray(movie_bias, dtype=np.float32)

    nr = N_ROWS
    ublk = np.zeros((nr, M, K + 2), np.float32)
    ublk[:, :, :K] = taste_emb[:nr].reshape(nr, M, K)
    ublk[:, :, K] = user_bias[:nr, 0][:, None]
    ublk[:, :, K + 1] = 1.0
    user_packed = np.zeros((nr, UROW), np.float32)
    user_packed[:, 0:256] = attn_emb[:nr]
    user_packed[:, 256:528] = ublk.reshape(nr, 272)

    nm = movie_emb.shape[0]
    assert nm <= N_ROWS
    movie_packed = np.zeros((N_ROWS, VROW), np.float32)
    movie_packed[:nm, :K] = movie_emb
    movie_packed[:nm, K] = 1.0
    movie_packed[:nm, K + 1] = movie_bias[:, 0]
    return (
        user_packed.astype(ml_dtypes.bfloat16),
        movie_packed.astype(ml_dtypes.bfloat16),
    )


def wrap_idx(idx_logical: np.ndarray) -> np.ndarray:
    """dma_gather idx layout for ONE gather: [128, n/16] int16
    (16-partition wrap, replicated x8)."""
    n = idx_logical.shape[0]
    w = idx_logical.astype(np.int16).reshape(n // 16, 16).T  # [16, n/16]
    return np.tile(w, (P // 16, 1))


def group_core_edges(u, v, eidx):
    """Group one core's edges by user into G-slot groups.

    Returns (group_user [NGROUPS], slot_v [NGROUPS, G], slot_edge
    [NGROUPS, G] with -1 for dummy slots).  Group j is computed by
    (partition j%128, chunk j//128).
    """
    ngroups = CAP // G
    order = np.argsort(u, kind="stable")
    u_s, v_s, e_s = u[order], v[order], eidx[order]
    # segment boundaries per user
    bounds = np.flatnonzero(np.diff(u_s)) + 1
    starts = np.concatenate([[0], bounds])
    ends = np.concatenate([bounds, [len(u_s)]])

    group_user = np.full(ngroups, u[0] if len(u) else 0, dtype=np.int64)
    slot_v = np.zeros((ngroups, G), dtype=np.int64)
    slot_edge = np.full((ngroups, G), -1, dtype=np.int64)
    gj = 0
    for s, e in zip(starts, ends):
        for base in range(s, e, G):
            take = min(G, e - base)
            assert gj < ngroups, "CAP too small for this edge distribution"
            group_user[gj] = u_s[s]
            slot_v[gj, :take] = v_s[base : base + take]
            slot_edge[gj, :take] = e_s[base : base + take]
            gj += 1
    return group_user, slot_v, slot_edge


def prepare(edge, taste_emb, attn_emb, movie_emb, user_bias, movie_bias):
    edge = np.asarray(edge)
    u = edge[:, 0].astype(np.int64)
    v = edge[:, 1].astype(np.int64)
    b = edge.shape[0]
    assert b == B
    assert u.max() < N_ROWS and v.max() < N_ROWS

    user_packed, movie_packed = pack_tables(
        taste_emb, attn_emb, movie_emb, user_bias, movie_bias
    )

    core_of = u // UPC  # user-range partitioning
    uw, vw = GPC // 16, CHUNK // 16

    in_maps = []
    slot_edge_all = []
    for r in range(N_CORES):
        sel = np.flatnonzero(core_of == r)
        gu, sv, se = group_core_edges(u[sel], v[sel], sel)
        slot_edge_all.append(se)

        # group j -> (partition j%128, chunk j//128).  User gather sc covers
        # groups j in [sc*GPC, (sc+1)*GPC): logical gather position i ->
        # partition i%128, block i//128 = cc; so position i = group
        # (sc*G + i//128)*128 + i%128.
        gu_by_chunkpart = gu.reshape(N_CHUNKS, P)  # [chunk, partition]
        uparts = []
        for sc in range(N_SC):
            blk = gu_by_chunkpart[sc * G : (sc + 1) * G]  # [G(cc), P]
            uparts.append(wrap_idx(blk.reshape(-1)))
        # movie gather for chunk c: position i -> partition i%128, slot i//128
        # = slot s of group j = c*128 + i%128
        sv_by = sv.reshape(N_CHUNKS, P, G)  # [chunk, partition(j%128), slot]
        vparts = []
        for c in range(N_CHUNKS):
            vparts.append(wrap_idx(sv_by[c].T.reshape(-1)))  # (s p) order
        idx_uv = np.concatenate(uparts + vparts, axis=1)
        assert idx_uv.shape == (P, N_SC * uw + N_CHUNKS * vw)
        in_maps.append(
            {
                "user_packed": user_packed,
                "movie_packed": movie_packed,
                "idx_uv": idx_uv,
            }
        )
    return in_maps, slot_edge_all


_NC_CACHE: list = []


def run(in_maps, **kwargs):
    if not _NC_CACHE:
        _NC_CACHE.append(build_nc())
    return run_bass_kernel_spmd(
        _NC_CACHE[0], in_maps, core_ids=list(range(N_CORES)), **kwargs
    )


def unscatter(res, slot_edge_all):
    y = np.empty(B, dtype=np.float32)
    filled = 0
    for r in range(N_CORES):
        yc = res.results[r]["y"]  # [P, COLS]
        se = slot_edge_all[r]  # [NGROUPS, G]
        # slot s of group j -> yc[j%128, (j//128)*G + s]
        ngroups = se.shape[0]
        j = np.arange(ngroups)
        part = (j % P)[:, None]
        col = ((j // P) * G)[:, None] + np.arange(G)[None, :]
        vals = yc[part, col]  # [NGROUPS, G]
        mask = se >= 0
        y[se[mask]] = vals[mask]
        filled += int(mask.sum())
    assert filled == B
    return y


def kernel(edge, taste_emb, attn_emb, movie_emb, user_bias, movie_bias):
    in_maps, slot_edge_all = prepare(
        edge, taste_emb, attn_emb, movie_emb, user_bias, movie_bias
    )
    res = run(in_maps)
    return unscatter(res, slot_edge_all)


# revision 21
# speedup vs baseline: 1.4887x; 1.0558x over previous
"""Trainium2 Bass kernel for the mixture-of-tastes edge scoring model.

y[b] = sum_m softmax_m(A[u_b] @ e[v_b]) * (U[u_b] @ e[v_b]) + ub[u_b] + mb[v_b]

The kernel is gather-descriptor-bound on TRN2 (the Q7 SWDGE generates
descriptors at ~8-10 ns each), so the layout is built to minimize
descriptor count:

- Edges are partitioned across the 8 cores BY USER RANGE (user u goes to
  core u // 2500), so each user's ~26 edges land on one core.  Each core's
  edges are grouped by user into groups of G=8 slots (padded with dummy
  slots), so ONE user-row gather descriptor serves 8 edges.
- Movie rows are gathered per slot (unavoidable: 1 descriptor each).
- Group j maps to (partition j%128, output column block j//128); slot s of
  group j is output element [j%128, (j//128)*8 + s].  The host keeps a
  slot->edge map and unscatters at the end (dummy slots dropped).

Tables are packed on the host into gather-friendly bf16 rows (bf16 also
gives the DVE its 2x 16-bit mode):

  user_packed[u]  = [attn(8x32) | taste'(8x34) | pad]  (640 bf16 = 1280 B)
      taste'[m] = [taste[m] (32) | user_bias[u] | 1.0]
  movie_packed[v] = [e (32) | 1.0 | mb | pad]          (128 bf16 = 256 B)

With e'' = movie_packed[v][0:34] = [e, 1, mb], the fold
  U'_m . e'' = U_m . e + ub + mb
adds (ub+mb) to every score; softmax weights sum to 1, so the output gets
+(ub+mb) with no separate bias gather.  Softmax is computed without max
subtraction (logits are O(1e-2) here; exp cannot overflow).

Per 1024-slot chunk: one movie dma_gather + DVE broadcast-multiply
(user rows broadcast over the 8 slots of their group) + 3D-AP reduces,
ACT exp, DVE weighted combine.  One 1024-group user dma_gather feeds 8
chunks.
"""

import sys

sys.path.insert(0, "/opt/trn_rl_repo")

import ml_dtypes
import numpy as np

import concourse.bacc as bacc
import concourse.bass as bass
import concourse.mybir as mybir
from concourse.bass_utils import run_bass_kernel_spmd
from concourse.tile import TileContext

# Problem constants (nn_MoT_43533788512463)
B = 524288
N_CORES = 8
M, K = 8, 32
N_ROWS = 20000  # edge indices are randint(0, 20000) per the spec
UPC = N_ROWS // N_CORES  # users per core (u-range partitioning)
G = 8  # slots (edges) per user group
UROW = 640  # packed user row bf16: 256 attn + 272 taste' + 112 pad
VROW = 128  # packed movie row bf16: 32 e + 1.0 + mb + 94 pad
P = 128
CHUNK = 1024  # slots per movie gather / compute chunk
NBLK = CHUNK // P  # 8 column blocks per chunk

# Per-core slot capacity.  Expected need: 2500 users x E[ceil(n/8)] groups
# ~= 9570 +- 25; 9728 groups (76 chunks) is >6 sigma of slack.
N_CHUNKS = 76
CAP = N_CHUNKS * CHUNK  # 77824 slots
GPC = 512  # groups per user gather (512 groups = 4 chunks)
SC_CHUNKS = GPC * G // CHUNK  # 4 chunks per user super-chunk
N_SC = N_CHUNKS // SC_CHUNKS  # 19 user gathers
COLS = CAP // P  # 608 output columns per partition

BF16 = mybir.dt.bfloat16
F32 = mybir.dt.float32
I16 = mybir.dt.int16
MULT = mybir.AluOpType.mult
ADD = mybir.AluOpType.add
AX_X = mybir.AxisListType.X


def build_nc() -> bass.Bass:
    """One NeuronCore's program; SPMD across cores with different inputs."""
    nc = bacc.Bacc("TRN2", debug=False)
    user_d = nc.dram_tensor("user_packed", [N_ROWS, UROW], BF16, kind="ExternalInput")
    movie_d = nc.dram_tensor("movie_packed", [N_ROWS, VROW], BF16, kind="ExternalInput")
    # user idx: N_SC gathers x (GPC/16) cols; movie idx: N_CHUNKS x (CHUNK/16)
    uw, vw = GPC // 16, CHUNK // 16
    idx_d = nc.dram_tensor(
        "idx_uv", [P, N_SC * uw + N_CHUNKS * vw], I16, kind="ExternalInput"
    )
    y_d = nc.dram_tensor("y", [P, COLS], F32, kind="ExternalOutput")

    with TileContext(nc) as tc:
        with (
            tc.tile_pool(name="persist", bufs=1) as pp,
            tc.tile_pool(name="io", bufs=3) as iop,
            tc.tile_pool(name="mid", bufs=2) as midp,
        ):
            idxs = pp.tile([P, N_SC * uw + N_CHUNKS * vw], I16)
            nc.sync.dma_start(idxs[:, :], idx_d[:, :])
            ysb = pp.tile([P, COLS], F32)

            for sc in range(N_SC):
                us = iop.tile([P, SC_CHUNKS, UROW], BF16, tag="us")
                usl = idxs[:, sc * uw : (sc + 1) * uw]
                nc.gpsimd.dma_gather(
                    us[:, :, :], user_d[:, :], usl, GPC, GPC, UROW
                )
                for cc in range(SC_CHUNKS):
                    c = sc * SC_CHUNKS + cc
                    mv = iop.tile([P, NBLK, VROW], BF16, tag="mv")
                    vsl = idxs[
                        :, N_SC * uw + c * vw : N_SC * uw + (c + 1) * vw
                    ]
                    nc.gpsimd.dma_gather(
                        mv[:, :, :], movie_d[:, :], vsl, CHUNK, CHUNK, VROW
                    )

                    # group's user row broadcast over its 8 slots (dim 1);
                    # slot's movie row broadcast over the 8 tastes (dim 2)
                    a4 = (
                        us[:, cc, 0:256]
                        .rearrange("p (m k) -> p m k", m=M)
                        .unsqueeze(1)
                        .broadcast_to([P, NBLK, M, K])
                    )
                    u4 = (
                        us[:, cc, 256:528]
                        .rearrange("p (m k) -> p m k", m=M)
                        .unsqueeze(1)
                        .broadcast_to([P, NBLK, M, K + 2])
                    )
                    e32 = (
                        mv[:, :, 0:K].unsqueeze(2).broadcast_to([P, NBLK, M, K])
                    )
                    e34 = (
                        mv[:, :, 0 : K + 2]
                        .unsqueeze(2)
                        .broadcast_to([P, NBLK, M, K + 2])
                    )

                    prod_a = midp.tile([P, NBLK, M, K], BF16, tag="prod_a")
                    prod_u = midp.tile([P, NBLK, M, K + 2], BF16, tag="prod_u")
                    half_a = midp.tile([P, NBLK, M, K // 2], BF16, tag="half_a")
                    half_u = midp.tile([P, NBLK, M, K // 2 + 1], BF16, tag="half_u")
                    logits = midp.tile([P, NBLK, M], F32, tag="logits")
                    scores = midp.tile([P, NBLK, M], F32, tag="scores")
                    exps = midp.tile([P, NBLK, M], F32, tag="exps")
                    wprod = midp.tile([P, NBLK, M], F32, tag="wprod")
                    num_t = midp.tile([P, NBLK], F32, tag="num_t")
                    den_t = midp.tile([P, NBLK], F32, tag="den_t")
                    rden_t = midp.tile([P, NBLK], F32, tag="rden_t")

                    # mul at bf16 2x; fold k in half with a bf16 add (2x)
                    # before tensor_reduce, which only has a 1x uop
                    nc.vector.tensor_tensor(prod_a[:, :, :, :], a4, e32, op=MULT)
                    nc.vector.tensor_tensor(
                        half_a[:, :, :, :],
                        prod_a[:, :, :, 0 : K // 2],
                        prod_a[:, :, :, K // 2 : K],
                        op=ADD,
                    )
                    nc.vector.tensor_reduce(
                        logits[:, :, :], half_a[:, :, :, :], AX_X, ADD
                    )
                    nc.vector.tensor_tensor(prod_u[:, :, :, :], u4, e34, op=MULT)
                    nc.vector.tensor_tensor(
                        half_u[:, :, :, :],
                        prod_u[:, :, :, 0 : K // 2 + 1],
                        prod_u[:, :, :, K // 2 + 1 : K + 2],
                        op=ADD,
                    )
                    nc.vector.tensor_reduce(
                        scores[:, :, :], half_u[:, :, :, :], AX_X, ADD
                    )
                    nc.scalar.activation(
                        exps[:, :, :],
                        logits[:, :, :],
                        mybir.ActivationFunctionType.Exp,
                    )
                    nc.vector.tensor_tensor(
                        wprod[:, :, :], exps[:, :, :], scores[:, :, :], op=MULT
                    )
                    nc.vector.tensor_reduce(num_t[:, :], wprod[:, :, :], AX_X, ADD)
                    nc.vector.tensor_reduce(den_t[:, :], exps[:, :, :], AX_X, ADD)
                    nc.vector.reciprocal(rden_t[:, :], den_t[:, :])
                    nc.vector.tensor_tensor(
                        ysb[:, c * NBLK : (c + 1) * NBLK],
                        num_t[:, :],
                        rden_t[:, :],
                        op=MULT,
                    )

            nc.sync.dma_start(y_d[:, :], ysb[:, :])

    nc.compile()
    return nc


def pack_tables(taste_emb, attn_emb, movie_emb, user_bias, movie_bias):
    taste_emb = np.asarray(taste_emb, dtype=np.float32)
    attn_emb = np.asarray(attn_emb, dtype=np.float32)
    movie_emb = np.asarray(movie_emb, dtype=np.float32)
    user_bias = np.asarray(user_bias, dtype=np.float32)
    movie_bias = np.asarray(movie_bias, dtype=np.float32)

    nr = N_ROWS
    ublk = np.zeros((nr, M, K + 2), np.float32)
    ublk[:, :, :K] = taste_emb[:nr].reshape(nr, M, K)
    ublk[:, :, K] = user_bias[:nr, 0][:, None]
    ublk[:, :, K + 1] = 1.0
    user_packed = np.zeros((nr, UROW), np.float32)
    user_packed[:, 0:256] = attn_emb[:nr]
    user_packed[:, 256:528] = ublk.reshape(nr, 272)

    nm = movie_emb.shape[0]
    assert nm <= N_ROWS
    movie_packed = np.zeros((N_ROWS, VROW), np.float32)
    movie_packed[:nm, :K] = movie_emb
    movie_packed[:nm, K] = 1.0
    movie_packed[:nm, K + 1] = movie_bias[:, 0]
    return (
        user_packed.astype(ml_dtypes.bfloat16),
        movie_packed.astype(ml_dtypes.bfloat16),
    )


def wrap_idx(idx_logical: np.ndarray) -> np.ndarray:
    """dma_gather idx layout for ONE gather: [128, n/16] int16
    (16-partition wrap, replicated x8)."""
    n = idx_logical.shape[0]
    w = idx_logical.astype(np.int16).reshape(n // 16, 16).T  # [16, n/16]
    return np.tile(w, (P // 16, 1))


def group_core_edges(u, v, eidx):
    """Group one core's edges by user into G-slot groups.

    Returns (group_user [NGROUPS], slot_v [NGROUPS, G], slot_edge
    [NGROUPS, G] with -1 for dummy slots).  Group j is computed by
    (partition j%128, chunk j//128).
    """
    ngroups = CAP // G
    order = np.argsort(u, kind="stable")
    u_s, v_s, e_s = u[order], v[order], eidx[order]
    # segment boundaries per user
    bounds = np.flatnonzero(np.diff(u_s)) + 1
    starts = np.concatenate([[0], bounds])
    ends = np.concatenate([bounds, [len(u_s)]])

    group_user = np.full(ngroups, u[0] if len(u) else 0, dtype=np.int64)
    slot_v = np.zeros((ngroups, G), dtype=np.int64)
    slot_edge = np.full((ngroups, G), -1, dtype=np.int64)
    gj = 0
    for s, e in zip(starts, ends):
        for base in range(s, e, G):
            take = min(G, e - base)
            assert gj < ngroups, "CAP too small for this edge distribution"
            group_user[gj] = u_s[s]
            slot_v[gj, :take] = v_s[base : base + take]
            slot_edge[gj, :take] = e_s[base : base + take]
            gj += 1
    return group_user, slot_v, slot_edge


def prepare(edge, taste_emb, attn_emb, movie_emb, user_bias, movie_bias):
    edge = np.asarray(edge)
    u = edge[:, 0].astype(np.int64)
    v = edge[:, 1].astype(np.int64)
    b = edge.shape[0]
    assert b == B
    assert u.max() < N_ROWS and v.max() < N_ROWS

    user_packed, movie_packed = pack_tables(
        taste_emb, attn_emb, movie_emb, user_bias, movie_bias
    )

    core_of = u // UPC  # user-range partitioning
    uw, vw = GPC // 16, CHUNK // 16

    in_maps = []
    slot_edge_all = []
    for r in range(N_CORES):
        sel = np.flatnonzero(core_of == r)
        gu, sv, se = group_core_edges(u[sel], v[sel], sel)
        slot_edge_all.append(se)

        # group j -> (partition j%128, chunk j//128).  User gather sc covers
        # groups j in [sc*GPC, (sc+1)*GPC): logical gather position i ->
        # partition i%128, block i//128 = cc; so position i = group
        # (sc*G + i//128)*128 + i%128.
        gu_by_chunkpart = gu.reshape(N_CHUNKS, P)  # [chunk, partition]
        uparts = []
        for sc in range(N_SC):
            blk = gu_by_chunkpart[
                sc * SC_CHUNKS : (sc + 1) * SC_CHUNKS
            ]  # [SC_CHUNKS(cc), P]
            uparts.append(wrap_idx(blk.reshape(-1)))
        # movie gather for chunk c: position i -> partition i%128, slot i//128
        # = slot s of group j = c*128 + i%128
        sv_by = sv.reshape(N_CHUNKS, P, G)  # [chunk, partition(j%128), slot]
        vparts = []
        for c in range(N_CHUNKS):
            vparts.append(wrap_idx(sv_by[c].T.reshape(-1)))  # (s p) order
        idx_uv = np.concatenate(uparts + vparts, axis=1)
        assert idx_uv.shape == (P, N_SC * uw + N_CHUNKS * vw)
        in_maps.append(
            {
                "user_packed": user_packed,
                "movie_packed": movie_packed,
                "idx_uv": idx_uv,
            }
        )
    return in_maps, slot_edge_all


_NC_CACHE: list = []


def run(in_maps, **kwargs):
    if not _NC_CACHE:
        _NC_CACHE.append(build_nc())
    return run_bass_kernel_spmd(
        _NC_CACHE[0], in_maps, core_ids=list(range(N_CORES)), **kwargs
    )


def unscatter(res, slot_edge_all):
    y = np.empty(B, dtype=np.float32)
    filled = 0
    for r in range(N_CORES):
        yc = res.results[r]["y"]  # [P, COLS]
        se = slot_edge_all[r]  # [NGROUPS, G]
        # slot s of group j -> yc[j%128, (j//128)*G + s]
        ngroups = se.shape[0]
        j = np.arange(ngroups)
        part = (j % P)[:, None]
        col = ((j // P) * G)[:, None] + np.arange(G)[None, :]
        vals = yc[part, col]  # [NGROUPS, G]
        mask = se >= 0
        y[se[mask]] = vals[mask]
        filled += int(mask.sum())
    assert filled == B
    return y


def kernel(edge, taste_emb, attn_emb, movie_emb, user_bias, movie_bias):
    in_maps, slot_edge_all = prepare(
        edge, taste_emb, attn_emb, movie_emb, user_bias, movie_bias
    )
    res = run(in_maps)
    return unscatter(res, slot_edge_all)
